# revision 39
# baseline (speedup 1.0000x reference)
"""GNN message-passing NodeBlock kernel for 8 Trainium2 NeuronCores.

Problem:
    agg_a = segment_sum(edata_a, conn_a[1], 100000)   # [N, 64]
    agg_b = segment_sum(edata_b, conn_b[1], 100000)   # [N, 64]
    out   = concat([agg_a, agg_b, vdata], 1) @ W + b  # [N, 128]

Sharding: edges sharded BY RECEIVER. Nodes are packed on the host into
1568 windows of exactly 64 nodes chosen so that each window's edge
counts land just under 512 for BOTH types (degree-aware bucket-pool
packing: pick nodes at the needed-degree rate, never overshoot; ~1.1%
tile padding vs 13.6% for fixed node-range windows) - so nearly every
window is exactly 4 a-tiles + 4 b-tiles and the per-step SPMD padding
vanishes. Windows are snake-assigned to the 8 cores in descending
tile-count order; the host reassembles the output by explicit node
lists, so each core computes its windows' aggregation completely
locally (no collective). Each 128-edge tile is scattered into its
64-node window via a one-hot selection matrix (is_equal of rel vs
iota) and a PE matmul accumulated in PSUM; the dense updater follows
as 512-col matmuls, software-pipelined ONE BLOCK LATE (vt half first)
so the PE never waits on the Act-engine PSUM->SBUF x0 copy.

Precision (gate is 2e-2): edge features and vdata travel fp8-e3m4
(4 mantissa bits; e4m3 measured 2.7e-2 - fails; e3m4 measures
1.31e-2), W/x0/sel bf16, output stored bf16 and widened on host.

WIN=64 matches the PE floor: each 128-edge tile costs
max(64-col ldweights, WIN-col moving pass) on PE, so WIN<64 only
hurts (measured: WIN=32 ballooned LDWEIGHTS to 81us and exec to
126us). Measured engine busy at 88.8us exec: PE 66.5us (the wall,
~32ns/tile steady), DVE 55.8, DMA ~52, Act 32.7.

Data movement: edge tiles for a whole block arrive as ONE ~0.5 MB DMA
on the sync HWDGE queue (kept free of everything else); consts, vT,
rel and output stores ride the scalar queue. rel is split into a head
tile (first 6 blocks) + rest tile so the first sel builds do not wait
on the full-rel DMA (per-tile dependency tracking). The sel build
needs sel in (t,w) layout; the host stores rel DUPLICATED x2
(rel2[p,2t+d] = rel[p,t]) so the is_equal AP has a dense 64-bit
innermost run, which keeps DVE in its 2x uop mode. (GpSimd rejects
is_equal at codegen; scalar_tensor_tensor would reach the 4x uop but
walrus caps it at 3D access patterns - both measured dead ends.)

Schedule: a half-size block leads (fast pipeline fill), then full
blocks in descending window size; the final output store is a single
block so the drain tail stays short. Padding slots carry rel=-1 and
zero data.
"""
import numpy as np
import ml_dtypes

import concourse.bass as bass
import concourse.tile as tile
from concourse import mybir
from concourse.bass_utils import run_bass_kernel_spmd
from concourse.vector_clock import ScopedClock

BF16 = ml_dtypes.bfloat16
E3M4 = ml_dtypes.float8_e3m4

N_NODES = 100000
N_EDGES = 800000
D_EDGE = 64
D_NODE = 128
D_OUT = 128
N_CORES = 8
WIN = 64                   # nodes per window
WPC = 196                  # windows per core
NWIN = WPC * N_CORES       # 1568 global windows
NPC = WIN * WPC            # nodes per core (12544)
NTOT = NPC * N_CORES       # padded node space (100352)
TGT = 508                  # per-window per-type edge-count target (<=512)
BLK_STEPS = 8              # max windows per phase-2 block (8*64 = 512 cols)
# small 4-step block FIRST so the pipeline fills fast (its edge DMA and
# sel build are half-size); the remaining 24 full blocks follow
BLOCK_PLAN = [4] + [8] * 24
N_BLKS = len(BLOCK_PLAN)
REL_HEAD_BLKS = 6          # rel tile split: head covers first blocks
OUT_PLAN = (12, 8, 4, 1)   # blocks per outT store (tiny final store)

# ---------------------------------------------------------------------------
# compat patches for this container's walrus build
# ---------------------------------------------------------------------------

_MAX_WAITS = 1


def _patched_drain_and_barrier(self, tick_clock, wait_clock):
    nc = self.nc
    probe = nc.sync.nop(nofuse=True, hint="tile_drain_wait0")
    wait_clock.add_sem_waits(
        probe.ins, ScopedClock({None: tick_clock.global_clock})
    )
    si = probe.ins.sync_info
    waits = list(si.on_wait) if si is not None and si.on_wait else []
    if len(waits) > _MAX_WAITS:
        si.on_wait = waits[:_MAX_WAITS]
        for k in range(_MAX_WAITS, len(waits), _MAX_WAITS):
            n = nc.sync.nop(nofuse=True, hint=f"tile_drain_wait{k}")
            n.ins.sync_info = mybir.SyncInfo(
                on_wait=waits[k : k + _MAX_WAITS], on_update=[]
            )
    drain_inst = nc.sync.drain()
    wait_clock.add_sem_waits(
        drain_inst.ins, ScopedClock({None: tick_clock.global_clock})
    )
    dsi = drain_inst.ins.sync_info
    if dsi is not None and dsi.on_wait and len(dsi.on_wait) > _MAX_WAITS:
        dsi.on_wait = []
    nc.all_engine_barrier()
    assert self.sems is not None
    popped = nc._tile_sem_poison_stack.pop()
    assert popped is self._sem_poison
    nc.clear_and_free_semaphores(list(self.sems.allocated().values()))
    nc.all_engine_barrier()


def _split_multi_waits(nc):
    """This walrus build accepts one sync-wait per TPB instruction; move
    extra waits onto preceding same-engine NOPs."""
    for fn in nc.m.functions:
        for blk in fn.blocks:
            out = []
            changed = False
            for inst in blk.instructions:
                si = inst.sync_info
                if si is not None and si.on_wait and len(si.on_wait) > 1:
                    waits = list(si.on_wait)
                    for j, w in enumerate(waits[:-1]):
                        nop = mybir.InstNoOp(
                            name=f"{inst.name}_xw{j}", ins=[], outs=[]
                        )
                        nop.engine = inst.engine
                        nop.sync_info = mybir.SyncInfo(
                            on_wait=[w], on_update=[]
                        )
                        out.append(nop)
                    si.on_wait = [waits[-1]]
                    changed = True
                out.append(inst)
            if changed:
                blk.instructions = out


def _install_ntff_hook_shim():
    import sys
    import types

    if "antenv.axon_hooks" in sys.modules:
        return
    mod = types.ModuleType("antenv.axon_hooks")
    _hook = [None]
    mod.set_axon_ntff_profile_hook = lambda h: _hook.__setitem__(0, h)
    mod.get_axon_ntff_profile_hook = lambda: _hook[0]
    sys.modules["antenv.axon_hooks"] = mod
    try:
        import antenv

        antenv.axon_hooks = mod
    except ImportError:
        pass
    try:
        from trn_agent_boot.trn_boot import _ntff_profile_via_ctypes

        mod.set_axon_ntff_profile_hook(
            _ntff_profile_via_ctypes("/opt/axon/libaxon_pjrt.so")
        )
    except Exception:
        pass


tile.TileContext._drain_and_barrier = _patched_drain_and_barrier
_install_ntff_hook_shim()

# ---------------------------------------------------------------------------
# host-side window packing / sharding
# ---------------------------------------------------------------------------


def _pack_windows(deg_a, deg_b):
    """Partition NTOT node slots (real + zero-degree pad) into NWIN
    windows of exactly WIN nodes so each window's per-type edge sums
    land at/just under TGT+1. Returns [NWIN, WIN] node ids."""
    da = np.zeros(NTOT, np.int32)
    db = np.zeros(NTOT, np.int32)
    da[:N_NODES] = deg_a
    db[:N_NODES] = deg_b
    DMAX = int(max(da.max(), db.max())) + 1
    key = da.astype(np.int64) * DMAX + db
    order = np.argsort(key, kind="stable")
    skey = key[order]
    starts = np.searchsorted(skey, np.arange(DMAX * DMAX))
    ends = np.searchsorted(skey, np.arange(DMAX * DMAX) + 1)
    pos = starts.copy()
    avail = (ends - starts).astype(np.int64)
    navail = int(avail.sum())
    cand = [(i, j) for i in range(-DMAX, DMAX + 1)
            for j in range(-DMAX, DMAX + 1)]
    cand.sort(key=lambda x: x[0] * x[0] + x[1] * x[1])
    wins = np.full((NWIN, WIN), -1, np.int64)
    for w in range(NWIN):
        ra, rb = TGT, TGT
        k = 0
        while k < WIN and navail > 0:
            rem = WIN - k
            pi = min(max(int(round(max(ra, 0) / rem)), 0), DMAX - 1)
            qi = min(max(int(round(max(rb, 0) / rem)), 0), DMAX - 1)
            found = -1
            fallback = -1
            for (di, dj) in cand:
                i, j = pi + di, qi + dj
                if 0 <= i < DMAX and 0 <= j < DMAX and avail[i * DMAX + j] > 0:
                    if fallback < 0:
                        fallback = i * DMAX + j
                    if i <= ra and j <= rb:
                        found = i * DMAX + j
                        break
            b = found if found >= 0 else fallback
            node = order[pos[b]]
            pos[b] += 1
            avail[b] -= 1
            navail -= 1
            wins[w, k] = node
            ra -= da[node]
            rb -= db[node]
            k += 1
    return wins


def _preprocess(vdata, edata_a, edata_b, conn_a, conn_b, W_mat, b_vec):
    recv_a = np.asarray(conn_a[1]).astype(np.int64)
    recv_b = np.asarray(conn_b[1]).astype(np.int64)

    deg_a = np.bincount(recv_a, minlength=N_NODES)
    deg_b = np.bincount(recv_b, minlength=N_NODES)
    wins_nodes = _pack_windows(deg_a, deg_b)  # [NWIN, WIN]

    node2win = np.empty(NTOT, np.int64)
    node2rel = np.empty(NTOT, np.int64)
    flat = wins_nodes.reshape(-1)
    node2win[flat] = np.repeat(np.arange(NWIN), WIN)
    node2rel[flat] = np.tile(np.arange(WIN), NWIN)

    def bin_type(recv):
        gwin = node2win[recv]
        order = np.argsort(gwin, kind="stable")
        counts = np.bincount(gwin, minlength=NWIN)
        starts = np.zeros(NWIN + 1, dtype=np.int64)
        np.cumsum(counts, out=starts[1:])
        return order, counts, starts

    ids_a, cnt_a, st_a = bin_type(recv_a)
    ids_b, cnt_b, st_b = bin_type(recv_b)

    ta_g = np.maximum(1, np.ceil(cnt_a / 128)).astype(np.int64)  # [NWIN]
    tb_g = np.maximum(1, np.ceil(cnt_b / 128)).astype(np.int64)
    # snake-assign windows to cores in descending tile-count order: balances
    # per-core totals and aligns per-step order statistics so max-over-cores
    # SPMD padding nearly vanishes
    order = np.argsort(-(ta_g * 1000 + tb_g), kind="stable")
    gwins = np.empty((N_CORES, WPC), dtype=np.int64)
    for i, w in enumerate(order):
        r, k = divmod(i, N_CORES)
        c = k if r % 2 == 0 else N_CORES - 1 - k
        gwins[c, r] = w
    tiles_a = ta_g[gwins]  # [N_CORES, WPC]
    tiles_b = tb_g[gwins]
    na_step = np.maximum(tiles_a.max(axis=0), 1)  # [WPC]
    nb_step = np.maximum(tiles_b.max(axis=0), 1)

    # per-step slot offsets in the packed (a+b per block) layout
    step_off_a = np.zeros(WPC, np.int64)
    step_off_b = np.zeros(WPC, np.int64)
    blk_base = 0
    i0 = 0
    for j in range(N_BLKS):
        steps = BLOCK_PLAN[j]
        na_blk = int(na_step[i0 : i0 + steps].sum())
        o = blk_base
        for i in range(i0, i0 + steps):
            step_off_a[i] = o
            o += na_step[i]
        o = blk_base + na_blk
        for i in range(i0, i0 + steps):
            step_off_b[i] = o
            o += nb_step[i]
        blk_base = o
        i0 += steps
    T_tot = int(blk_base)

    ea8 = np.asarray(edata_a).astype(E3M4)
    eb8 = np.asarray(edata_b).astype(E3M4)

    vdata = np.asarray(vdata)
    vpad = np.zeros((NTOT, D_NODE), dtype=np.float32)
    vpad[:N_NODES] = vdata

    iota = np.ascontiguousarray(
        np.broadcast_to(np.arange(WIN, dtype=np.float32), (128, WIN))
    ).astype(BF16)
    Wf = np.ascontiguousarray(np.asarray(W_mat), dtype=np.float32).astype(BF16)
    bf = np.asarray(b_vec).astype(np.float32).reshape(D_OUT, 1)

    rel_a = node2rel[recv_a].astype(np.float32)
    rel_b = node2rel[recv_b].astype(np.float32)

    in_maps = []
    for c in range(N_CORES):
        slot_eid = np.full(T_tot * 128, -1, dtype=np.int64)
        slot_rel = np.full(T_tot * 128, -1.0, dtype=np.float32)
        slot_is_a = np.zeros(T_tot * 128, dtype=bool)
        for i in range(WPC):
            g = gwins[c][i]
            for ids, starts, cnts, soff, rel, is_a in (
                (ids_a, st_a, cnt_a, step_off_a, rel_a, True),
                (ids_b, st_b, cnt_b, step_off_b, rel_b, False),
            ):
                cnt = cnts[g]
                if cnt == 0:
                    continue
                eids = ids[starts[g] : starts[g] + cnt]
                s0 = soff[i] * 128
                slot_eid[s0 : s0 + cnt] = eids
                slot_is_a[s0 : s0 + cnt] = is_a
                slot_rel[s0 : s0 + cnt] = rel[eids]
        idx = np.maximum(slot_eid, 0)
        gath = np.where(slot_is_a[:, None], ea8[idx], eb8[idx])
        gath[slot_eid < 0] = 0
        eh = np.ascontiguousarray(
            gath.reshape(T_tot, 128, 64).transpose(1, 0, 2)
        )  # [slot, tile, feat] e3m4
        relT = slot_rel.reshape(T_tot, 128).T.astype(BF16)
        rel2 = np.ascontiguousarray(np.repeat(relT, 2, axis=1))  # [128, 2T]
        nodes = wins_nodes[gwins[c]].reshape(-1)
        vT = np.ascontiguousarray(vpad[nodes].T.astype(E3M4))  # [128, NPC]
        in_maps.append(
            {"eh": eh, "rel": rel2, "vT": vT, "Wd": Wf, "bd": bf,
             "iota": iota}
        )

    sched = (tuple(int(x) for x in na_step), tuple(int(x) for x in nb_step))
    node_lists = wins_nodes[gwins].reshape(N_CORES, NPC)
    return in_maps, sched, node_lists


# ---------------------------------------------------------------------------
# device kernel
# ---------------------------------------------------------------------------

_NC_CACHE = {}


def _build(sched):
    na_step, nb_step = sched
    f32 = mybir.dt.float32
    bf16 = mybir.dt.bfloat16
    fp8 = mybir.dt.float8e3

    # packed per-block layout: [a tiles | b tiles] per block
    blk_na = []
    blk_nb = []
    blk_i0 = []
    i0 = 0
    for j in range(N_BLKS):
        steps = BLOCK_PLAN[j]
        blk_i0.append(i0)
        blk_na.append(sum(na_step[i0 : i0 + steps]))
        blk_nb.append(sum(nb_step[i0 : i0 + steps]))
        i0 += steps
    blk_tot = [a + b for a, b in zip(blk_na, blk_nb)]
    max_blk = max(blk_tot)
    T_tot = sum(blk_tot)

    nc = bass.Bass(trn_type="TRN2")
    eh_d = nc.dram_tensor("eh", [128, T_tot, 64], fp8, kind="ExternalInput")
    rel_d = nc.dram_tensor("rel", [128, 2 * T_tot], bf16, kind="ExternalInput")
    vT_d = nc.dram_tensor("vT", [128, NPC], fp8, kind="ExternalInput")
    W_d = nc.dram_tensor("Wd", [2 * D_NODE, D_OUT], bf16, kind="ExternalInput")
    b_d = nc.dram_tensor("bd", [D_OUT, 1], f32, kind="ExternalInput")
    iota_d = nc.dram_tensor("iota", [128, WIN], bf16, kind="ExternalInput")
    outT_d = nc.dram_tensor("outT", [128, NPC], bf16, kind="ExternalOutput")

    with tile.TileContext(nc) as tc:
        with (
            tc.tile_pool(name="consts", bufs=1) as cb,
            tc.tile_pool(name="x0", bufs=3) as x0p,
            tc.tile_pool(name="edges", bufs=5) as ep,
            tc.tile_pool(name="sel", bufs=4) as sp,
            tc.tile_pool(name="out", bufs=2) as op,
            tc.tile_pool(name="psum1", bufs=4, space="PSUM") as pp1,
            tc.tile_pool(name="psum2", bufs=2, space="PSUM") as pp2,
        ):
            rel_head = 2 * sum(blk_tot[:REL_HEAD_BLKS])
            # issue order matters: iota + rel head gate the first sel build
            iota_sb = cb.tile([128, WIN], bf16)
            nc.scalar.dma_start(iota_sb[:], iota_d[:, :])
            relh_sb = cb.tile([128, rel_head], bf16, tag="relh")
            nc.scalar.dma_start(relh_sb[:], rel_d[:, :rel_head])
            w0_sb = cb.tile([128, D_OUT], bf16, tag="w0")
            nc.scalar.dma_start(w0_sb[:], W_d[0:128, :])
            w1_sb = cb.tile([128, D_OUT], bf16, tag="w1")
            nc.scalar.dma_start(w1_sb[:], W_d[128:256, :])
            b_sb = cb.tile([D_OUT, 1], f32, tag="b")
            nc.scalar.dma_start(b_sb[:], b_d[:, :])
            relr_sb = cb.tile([128, 2 * T_tot - rel_head], bf16, tag="relr")
            vt_sb = cb.tile([128, NPC], fp8, tag="vt")

            off = 0
            ot = None
            chunk_col0 = 0
            ot_cols = 0
            chunk_starts = set()
            s = 0
            for n in OUT_PLAN:
                chunk_starts.add(s)
                s += n
            x0_prev = None
            cols_prev = 0
            i0_prev = 0

            def updater(i, x0i, colsi, i0i):
                """Dense update for block i (x0/vt -> po -> ot -> maybe store).
                Emitted one block late so PE never waits on the Act x0 copy;
                the vt half goes first since it has no x0 dependency."""
                nonlocal ot, chunk_col0, ot_cols
                po = pp2.tile([128, BLK_STEPS * WIN], f32, tag="p2")
                nc.tensor.matmul(
                    out=po[:, :colsi],
                    lhsT=w1_sb[:],
                    rhs=vt_sb[:, i0i * WIN : i0i * WIN + colsi],
                    start=True, stop=False,
                )
                nc.tensor.matmul(
                    out=po[:, :colsi], lhsT=w0_sb[:], rhs=x0i[:, :colsi],
                    start=False, stop=True,
                )
                if i in chunk_starts:
                    ot = op.tile(
                        [128, max(OUT_PLAN) * BLK_STEPS * WIN], bf16, tag="ot"
                    )
                    chunk_col0 = i0i * WIN
                    ot_cols = 0
                nc.scalar.activation(
                    out=ot[:, ot_cols : ot_cols + colsi],
                    in_=po[:, :colsi],
                    func=mybir.ActivationFunctionType.Identity,
                    bias=b_sb[:, 0:1],
                    scale=1.0,
                )
                ot_cols += colsi
                if i + 1 in chunk_starts or i == N_BLKS - 1:
                    nc.scalar.dma_start(
                        outT_d[:, chunk_col0 : chunk_col0 + ot_cols],
                        ot[:, :ot_cols],
                    )

            for j in range(N_BLKS):
                i0 = blk_i0[j]
                steps = BLOCK_PLAN[j]
                cols_blk = steps * WIN
                n_blk = blk_tot[j]
                na_b = blk_na[j]

                # one coalesced edge DMA per block (~0.5 MB)
                et = ep.tile([128, max_blk * 64], fp8, tag="et")
                nc.sync.dma_start(
                    et[:, : n_blk * 64], eh_d[:, off : off + n_blk, :]
                )
                if j == 1:
                    nc.scalar.dma_start(relr_sb[:], rel_d[:, rel_head:])
                # vT arrives in 5 chunks spread over the early blocks
                if j < 10 and j % 2 == 0:
                    k = j // 2
                    vc0 = k * (NPC // 5)
                    vc1 = NPC if k == 4 else (k + 1) * (NPC // 5)
                    nc.scalar.dma_start(vt_sb[:, vc0:vc1], vT_d[:, vc0:vc1])

                selb = sp.tile([128, max_blk * WIN], bf16, tag="selb")
                # rel2 dup-x2 keeps a dense 64-bit innermost run -> 2x uop
                if j < REL_HEAD_BLKS:
                    rel_sb, rb0 = relh_sb, 2 * off
                else:
                    rel_sb, rb0 = relr_sb, 2 * off - rel_head
                in0 = rel_sb[:, rb0 : rb0 + 2 * n_blk].rearrange(
                    "p (n one d) -> p n one d", one=1, d=2
                ).broadcast_to([128, n_blk, WIN // 2, 2])
                in1 = iota_sb[:].rearrange(
                    "p (w d) -> p w d", d=2
                ).rearrange(
                    "p (one w) d -> p one w d", one=1
                ).broadcast_to([128, n_blk, WIN // 2, 2])
                # (scalar_tensor_tensor would reach the DVE 4x_2p uop, but
                # walrus limits it to 3D APs and this pattern needs 4)
                nc.vector.tensor_tensor(
                    out=selb[:, : n_blk * WIN].rearrange(
                        "p (n w d) -> p n w d", w=WIN // 2, d=2
                    ),
                    in0=in0, in1=in1, op=mybir.AluOpType.is_equal,
                )

                x0 = x0p.tile([128, BLK_STEPS * WIN], bf16, tag="x0")
                ps = pp1.tile([128, BLK_STEPS * WIN], f32, tag="p1")
                t = 0
                for half, n_stp in ((0, na_step), (1, nb_step)):
                    r0 = half * 64
                    tt = 0
                    n_half = blk_na[j] if half == 0 else blk_nb[j]
                    for stp in range(steps):
                        for k in range(n_stp[i0 + stp]):
                            nc.tensor.matmul(
                                out=ps[
                                    r0 : r0 + 64,
                                    stp * WIN : (stp + 1) * WIN,
                                ],
                                lhsT=et[:, t * 64 : (t + 1) * 64],
                                rhs=selb[:, t * WIN : (t + 1) * WIN],
                                start=(tt == 0),
                                stop=(tt == n_half - 1),
                            )
                            t += 1
                            tt += 1
                nc.scalar.copy(x0[:, :cols_blk], ps[:, :cols_blk])
                off += n_blk

                if x0_prev is not None:
                    updater(j - 1, x0_prev, cols_prev, i0_prev)
                x0_prev, cols_prev, i0_prev = x0, cols_blk, i0
            updater(N_BLKS - 1, x0_prev, cols_prev, i0_prev)
    _split_multi_waits(nc)
    return nc


# ---------------------------------------------------------------------------
# public entry point
# ---------------------------------------------------------------------------


def kernel(vdata, edata_a, edata_b, conn_a, conn_b, W, b, _trace=False):
    in_maps, sched, node_lists = _preprocess(
        vdata, edata_a, edata_b, conn_a, conn_b, W, b
    )
    nc = _NC_CACHE.get(sched)
    if nc is None:
        nc = _build(sched)
        _NC_CACHE[sched] = nc
    kwargs = {}
    if _trace:
        kwargs = dict(trace=True, trace_cores=[0])
    res = run_bass_kernel_spmd(
        nc, in_maps, core_ids=list(range(N_CORES)), **kwargs
    )

    out_full = np.empty((NTOT, D_OUT), dtype=np.float32)
    for c in range(N_CORES):
        outT = res.results[c]["outT"].astype(np.float32)  # [128, NPC] bf16
        out_full[node_lists[c]] = outT.T
    out = out_full[:N_NODES]
    if _trace:
        return out, res
    return out


# revision 40
# speedup vs baseline: 1.0044x; 1.0044x over previous
"""GNN message-passing NodeBlock kernel for 8 Trainium2 NeuronCores.

Problem:
    agg_a = segment_sum(edata_a, conn_a[1], 100000)   # [N, 64]
    agg_b = segment_sum(edata_b, conn_b[1], 100000)   # [N, 64]
    out   = concat([agg_a, agg_b, vdata], 1) @ W + b  # [N, 128]

Sharding: edges sharded BY RECEIVER. Nodes are packed on the host into
1568 windows of exactly 64 nodes chosen so that each window's edge
counts land just under 512 for BOTH types (degree-aware bucket-pool
packing: pick nodes at the needed-degree rate, never overshoot; ~1.1%
tile padding vs 13.6% for fixed node-range windows) - so nearly every
window is exactly 4 a-tiles + 4 b-tiles and the per-step SPMD padding
vanishes. Windows are snake-assigned to the 8 cores in descending
tile-count order; the host reassembles the output by explicit node
lists, so each core computes its windows' aggregation completely
locally (no collective). Each 128-edge tile is scattered into its
64-node window via a one-hot selection matrix (is_equal of rel vs
iota) and a PE matmul accumulated in PSUM; the dense updater follows
as 512-col matmuls, software-pipelined ONE BLOCK LATE (vt half first)
so the PE never waits on the Act-engine PSUM->SBUF x0 copy.

Precision (gate is 2e-2): edge features and vdata travel fp8-e3m4
(4 mantissa bits; e4m3 measured 2.7e-2 - fails; e3m4 measures
1.31e-2), W/x0/sel bf16, output stored bf16 and widened on host.

WIN=64 matches the PE floor: each 128-edge tile costs
max(64-col ldweights, WIN-col moving pass) on PE, so WIN<64 only
hurts (measured: WIN=32 ballooned LDWEIGHTS to 81us and exec to
126us). Measured engine busy at 88.8us exec: PE 66.5us (the wall,
~32ns/tile steady), DVE 55.8, DMA ~52, Act 32.7.

Data movement: edge tiles for a whole block arrive as ONE ~0.5 MB DMA
on the sync HWDGE queue (kept free of everything else); consts, vT,
rel and output stores ride the scalar queue. rel is split into a head
tile (first 6 blocks) + rest tile so the first sel builds do not wait
on the full-rel DMA (per-tile dependency tracking). The sel build
needs sel in (t,w) layout; the host stores rel DUPLICATED x2
(rel2[p,2t+d] = rel[p,t]) so the is_equal AP has a dense 64-bit
innermost run, which keeps DVE in its 2x uop mode. (GpSimd rejects
is_equal at codegen; scalar_tensor_tensor would reach the 4x uop but
walrus caps it at 3D access patterns - both measured dead ends.)

Schedule: a half-size block leads (fast pipeline fill), then full
blocks in descending window size; the final output store is a single
block so the drain tail stays short. Padding slots carry rel=-1 and
zero data.
"""
import numpy as np
import ml_dtypes

import concourse.bass as bass
import concourse.tile as tile
from concourse import mybir
from concourse.bass_utils import run_bass_kernel_spmd
from concourse.vector_clock import ScopedClock

BF16 = ml_dtypes.bfloat16
E3M4 = ml_dtypes.float8_e3m4

N_NODES = 100000
N_EDGES = 800000
D_EDGE = 64
D_NODE = 128
D_OUT = 128
N_CORES = 8
WIN = 64                   # nodes per window
WPC = 196                  # windows per core
NWIN = WPC * N_CORES       # 1568 global windows
NPC = WIN * WPC            # nodes per core (12544)
NTOT = NPC * N_CORES       # padded node space (100352)
TGT = 508                  # per-window per-type edge-count target (<=512)
BLK_STEPS = 8              # max windows per phase-2 block (8*64 = 512 cols)
# half-size blocks at BOTH ends: blocks 0+1 small so the pipeline fills
# fast (rel-head descriptors aren't stuck behind a full block-1 edge
# DMA), last block small so the x0-copy/updater/act/store tail is short
BLOCK_PLAN = [4, 4] + [8] * 23 + [4]
N_BLKS = len(BLOCK_PLAN)
REL_HEAD_BLKS = 6          # rel tile split: head covers first blocks
OUT_PLAN = (12, 8, 5, 1)   # blocks per outT store (tiny final store)

# ---------------------------------------------------------------------------
# compat patches for this container's walrus build
# ---------------------------------------------------------------------------

_MAX_WAITS = 1


def _patched_drain_and_barrier(self, tick_clock, wait_clock):
    nc = self.nc
    probe = nc.sync.nop(nofuse=True, hint="tile_drain_wait0")
    wait_clock.add_sem_waits(
        probe.ins, ScopedClock({None: tick_clock.global_clock})
    )
    si = probe.ins.sync_info
    waits = list(si.on_wait) if si is not None and si.on_wait else []
    if len(waits) > _MAX_WAITS:
        si.on_wait = waits[:_MAX_WAITS]
        for k in range(_MAX_WAITS, len(waits), _MAX_WAITS):
            n = nc.sync.nop(nofuse=True, hint=f"tile_drain_wait{k}")
            n.ins.sync_info = mybir.SyncInfo(
                on_wait=waits[k : k + _MAX_WAITS], on_update=[]
            )
    drain_inst = nc.sync.drain()
    wait_clock.add_sem_waits(
        drain_inst.ins, ScopedClock({None: tick_clock.global_clock})
    )
    dsi = drain_inst.ins.sync_info
    if dsi is not None and dsi.on_wait and len(dsi.on_wait) > _MAX_WAITS:
        dsi.on_wait = []
    nc.all_engine_barrier()
    assert self.sems is not None
    popped = nc._tile_sem_poison_stack.pop()
    assert popped is self._sem_poison
    nc.clear_and_free_semaphores(list(self.sems.allocated().values()))
    nc.all_engine_barrier()


def _split_multi_waits(nc):
    """This walrus build accepts one sync-wait per TPB instruction; move
    extra waits onto preceding same-engine NOPs."""
    for fn in nc.m.functions:
        for blk in fn.blocks:
            out = []
            changed = False
            for inst in blk.instructions:
                si = inst.sync_info
                if si is not None and si.on_wait and len(si.on_wait) > 1:
                    waits = list(si.on_wait)
                    for j, w in enumerate(waits[:-1]):
                        nop = mybir.InstNoOp(
                            name=f"{inst.name}_xw{j}", ins=[], outs=[]
                        )
                        nop.engine = inst.engine
                        nop.sync_info = mybir.SyncInfo(
                            on_wait=[w], on_update=[]
                        )
                        out.append(nop)
                    si.on_wait = [waits[-1]]
                    changed = True
                out.append(inst)
            if changed:
                blk.instructions = out


def _install_ntff_hook_shim():
    import sys
    import types

    if "antenv.axon_hooks" in sys.modules:
        return
    mod = types.ModuleType("antenv.axon_hooks")
    _hook = [None]
    mod.set_axon_ntff_profile_hook = lambda h: _hook.__setitem__(0, h)
    mod.get_axon_ntff_profile_hook = lambda: _hook[0]
    sys.modules["antenv.axon_hooks"] = mod
    try:
        import antenv

        antenv.axon_hooks = mod
    except ImportError:
        pass
    try:
        from trn_agent_boot.trn_boot import _ntff_profile_via_ctypes

        mod.set_axon_ntff_profile_hook(
            _ntff_profile_via_ctypes("/opt/axon/libaxon_pjrt.so")
        )
    except Exception:
        pass


tile.TileContext._drain_and_barrier = _patched_drain_and_barrier
_install_ntff_hook_shim()

# ---------------------------------------------------------------------------
# host-side window packing / sharding
# ---------------------------------------------------------------------------


def _pack_windows(deg_a, deg_b):
    """Partition NTOT node slots (real + zero-degree pad) into NWIN
    windows of exactly WIN nodes so each window's per-type edge sums
    land at/just under TGT+1. Returns [NWIN, WIN] node ids."""
    da = np.zeros(NTOT, np.int32)
    db = np.zeros(NTOT, np.int32)
    da[:N_NODES] = deg_a
    db[:N_NODES] = deg_b
    DMAX = int(max(da.max(), db.max())) + 1
    key = da.astype(np.int64) * DMAX + db
    order = np.argsort(key, kind="stable")
    skey = key[order]
    starts = np.searchsorted(skey, np.arange(DMAX * DMAX))
    ends = np.searchsorted(skey, np.arange(DMAX * DMAX) + 1)
    pos = starts.copy()
    avail = (ends - starts).astype(np.int64)
    navail = int(avail.sum())
    cand = [(i, j) for i in range(-DMAX, DMAX + 1)
            for j in range(-DMAX, DMAX + 1)]
    cand.sort(key=lambda x: x[0] * x[0] + x[1] * x[1])
    wins = np.full((NWIN, WIN), -1, np.int64)
    for w in range(NWIN):
        ra, rb = TGT, TGT
        k = 0
        while k < WIN and navail > 0:
            rem = WIN - k
            pi = min(max(int(round(max(ra, 0) / rem)), 0), DMAX - 1)
            qi = min(max(int(round(max(rb, 0) / rem)), 0), DMAX - 1)
            found = -1
            fallback = -1
            for (di, dj) in cand:
                i, j = pi + di, qi + dj
                if 0 <= i < DMAX and 0 <= j < DMAX and avail[i * DMAX + j] > 0:
                    if fallback < 0:
                        fallback = i * DMAX + j
                    if i <= ra and j <= rb:
                        found = i * DMAX + j
                        break
            b = found if found >= 0 else fallback
            node = order[pos[b]]
            pos[b] += 1
            avail[b] -= 1
            navail -= 1
            wins[w, k] = node
            ra -= da[node]
            rb -= db[node]
            k += 1
    return wins


def _preprocess(vdata, edata_a, edata_b, conn_a, conn_b, W_mat, b_vec):
    recv_a = np.asarray(conn_a[1]).astype(np.int64)
    recv_b = np.asarray(conn_b[1]).astype(np.int64)

    deg_a = np.bincount(recv_a, minlength=N_NODES)
    deg_b = np.bincount(recv_b, minlength=N_NODES)
    wins_nodes = _pack_windows(deg_a, deg_b)  # [NWIN, WIN]

    node2win = np.empty(NTOT, np.int64)
    node2rel = np.empty(NTOT, np.int64)
    flat = wins_nodes.reshape(-1)
    node2win[flat] = np.repeat(np.arange(NWIN), WIN)
    node2rel[flat] = np.tile(np.arange(WIN), NWIN)

    def bin_type(recv):
        gwin = node2win[recv]
        order = np.argsort(gwin, kind="stable")
        counts = np.bincount(gwin, minlength=NWIN)
        starts = np.zeros(NWIN + 1, dtype=np.int64)
        np.cumsum(counts, out=starts[1:])
        return order, counts, starts

    ids_a, cnt_a, st_a = bin_type(recv_a)
    ids_b, cnt_b, st_b = bin_type(recv_b)

    ta_g = np.maximum(1, np.ceil(cnt_a / 128)).astype(np.int64)  # [NWIN]
    tb_g = np.maximum(1, np.ceil(cnt_b / 128)).astype(np.int64)
    # snake-assign windows to cores in descending tile-count order: balances
    # per-core totals and aligns per-step order statistics so max-over-cores
    # SPMD padding nearly vanishes
    order = np.argsort(-(ta_g * 1000 + tb_g), kind="stable")
    gwins = np.empty((N_CORES, WPC), dtype=np.int64)
    for i, w in enumerate(order):
        r, k = divmod(i, N_CORES)
        c = k if r % 2 == 0 else N_CORES - 1 - k
        gwins[c, r] = w
    tiles_a = ta_g[gwins]  # [N_CORES, WPC]
    tiles_b = tb_g[gwins]
    na_step = np.maximum(tiles_a.max(axis=0), 1)  # [WPC]
    nb_step = np.maximum(tiles_b.max(axis=0), 1)

    # per-step slot offsets in the packed (a+b per block) layout
    step_off_a = np.zeros(WPC, np.int64)
    step_off_b = np.zeros(WPC, np.int64)
    blk_base = 0
    i0 = 0
    for j in range(N_BLKS):
        steps = BLOCK_PLAN[j]
        na_blk = int(na_step[i0 : i0 + steps].sum())
        o = blk_base
        for i in range(i0, i0 + steps):
            step_off_a[i] = o
            o += na_step[i]
        o = blk_base + na_blk
        for i in range(i0, i0 + steps):
            step_off_b[i] = o
            o += nb_step[i]
        blk_base = o
        i0 += steps
    T_tot = int(blk_base)

    ea8 = np.asarray(edata_a).astype(E3M4)
    eb8 = np.asarray(edata_b).astype(E3M4)

    vdata = np.asarray(vdata)
    vpad = np.zeros((NTOT, D_NODE), dtype=np.float32)
    vpad[:N_NODES] = vdata

    iota = np.ascontiguousarray(
        np.broadcast_to(np.arange(WIN, dtype=np.float32), (128, WIN))
    ).astype(BF16)
    Wf = np.ascontiguousarray(np.asarray(W_mat), dtype=np.float32).astype(BF16)
    bf = np.asarray(b_vec).astype(np.float32).reshape(D_OUT, 1)

    rel_a = node2rel[recv_a].astype(np.float32)
    rel_b = node2rel[recv_b].astype(np.float32)

    in_maps = []
    for c in range(N_CORES):
        slot_eid = np.full(T_tot * 128, -1, dtype=np.int64)
        slot_rel = np.full(T_tot * 128, -1.0, dtype=np.float32)
        slot_is_a = np.zeros(T_tot * 128, dtype=bool)
        for i in range(WPC):
            g = gwins[c][i]
            for ids, starts, cnts, soff, rel, is_a in (
                (ids_a, st_a, cnt_a, step_off_a, rel_a, True),
                (ids_b, st_b, cnt_b, step_off_b, rel_b, False),
            ):
                cnt = cnts[g]
                if cnt == 0:
                    continue
                eids = ids[starts[g] : starts[g] + cnt]
                s0 = soff[i] * 128
                slot_eid[s0 : s0 + cnt] = eids
                slot_is_a[s0 : s0 + cnt] = is_a
                slot_rel[s0 : s0 + cnt] = rel[eids]
        idx = np.maximum(slot_eid, 0)
        gath = np.where(slot_is_a[:, None], ea8[idx], eb8[idx])
        gath[slot_eid < 0] = 0
        eh = np.ascontiguousarray(
            gath.reshape(T_tot, 128, 64).transpose(1, 0, 2)
        )  # [slot, tile, feat] e3m4
        relT = slot_rel.reshape(T_tot, 128).T.astype(BF16)
        rel2 = np.ascontiguousarray(np.repeat(relT, 2, axis=1))  # [128, 2T]
        nodes = wins_nodes[gwins[c]].reshape(-1)
        vT = np.ascontiguousarray(vpad[nodes].T.astype(E3M4))  # [128, NPC]
        in_maps.append(
            {"eh": eh, "rel": rel2, "vT": vT, "Wd": Wf, "bd": bf,
             "iota": iota}
        )

    sched = (tuple(int(x) for x in na_step), tuple(int(x) for x in nb_step))
    node_lists = wins_nodes[gwins].reshape(N_CORES, NPC)
    return in_maps, sched, node_lists


# ---------------------------------------------------------------------------
# device kernel
# ---------------------------------------------------------------------------

_NC_CACHE = {}


def _build(sched):
    na_step, nb_step = sched
    f32 = mybir.dt.float32
    bf16 = mybir.dt.bfloat16
    fp8 = mybir.dt.float8e3

    # packed per-block layout: [a tiles | b tiles] per block
    blk_na = []
    blk_nb = []
    blk_i0 = []
    i0 = 0
    for j in range(N_BLKS):
        steps = BLOCK_PLAN[j]
        blk_i0.append(i0)
        blk_na.append(sum(na_step[i0 : i0 + steps]))
        blk_nb.append(sum(nb_step[i0 : i0 + steps]))
        i0 += steps
    blk_tot = [a + b for a, b in zip(blk_na, blk_nb)]
    max_blk = max(blk_tot)
    T_tot = sum(blk_tot)

    nc = bass.Bass(trn_type="TRN2")
    eh_d = nc.dram_tensor("eh", [128, T_tot, 64], fp8, kind="ExternalInput")
    rel_d = nc.dram_tensor("rel", [128, 2 * T_tot], bf16, kind="ExternalInput")
    vT_d = nc.dram_tensor("vT", [128, NPC], fp8, kind="ExternalInput")
    W_d = nc.dram_tensor("Wd", [2 * D_NODE, D_OUT], bf16, kind="ExternalInput")
    b_d = nc.dram_tensor("bd", [D_OUT, 1], f32, kind="ExternalInput")
    iota_d = nc.dram_tensor("iota", [128, WIN], bf16, kind="ExternalInput")
    outT_d = nc.dram_tensor("outT", [128, NPC], bf16, kind="ExternalOutput")

    with tile.TileContext(nc) as tc:
        with (
            tc.tile_pool(name="consts", bufs=1) as cb,
            tc.tile_pool(name="x0", bufs=3) as x0p,
            tc.tile_pool(name="edges", bufs=5) as ep,
            tc.tile_pool(name="sel", bufs=4) as sp,
            tc.tile_pool(name="out", bufs=2) as op,
            tc.tile_pool(name="psum1", bufs=4, space="PSUM") as pp1,
            tc.tile_pool(name="psum2", bufs=2, space="PSUM") as pp2,
        ):
            rel_head = 2 * sum(blk_tot[:REL_HEAD_BLKS])
            # issue order matters: iota + rel head gate the first sel build
            iota_sb = cb.tile([128, WIN], bf16)
            nc.scalar.dma_start(iota_sb[:], iota_d[:, :])
            relh_sb = cb.tile([128, rel_head], bf16, tag="relh")
            nc.scalar.dma_start(relh_sb[:], rel_d[:, :rel_head])
            w0_sb = cb.tile([128, D_OUT], bf16, tag="w0")
            nc.scalar.dma_start(w0_sb[:], W_d[0:128, :])
            w1_sb = cb.tile([128, D_OUT], bf16, tag="w1")
            nc.scalar.dma_start(w1_sb[:], W_d[128:256, :])
            b_sb = cb.tile([D_OUT, 1], f32, tag="b")
            nc.scalar.dma_start(b_sb[:], b_d[:, :])
            relr_sb = cb.tile([128, 2 * T_tot - rel_head], bf16, tag="relr")
            vt_sb = cb.tile([128, NPC], fp8, tag="vt")

            off = 0
            ot = None
            chunk_col0 = 0
            ot_cols = 0
            chunk_starts = set()
            s = 0
            for n in OUT_PLAN:
                chunk_starts.add(s)
                s += n
            x0_prev = None
            cols_prev = 0
            i0_prev = 0

            def updater(i, x0i, colsi, i0i):
                """Dense update for block i (x0/vt -> po -> ot -> maybe store).
                Emitted one block late so PE never waits on the Act x0 copy;
                the vt half goes first since it has no x0 dependency."""
                nonlocal ot, chunk_col0, ot_cols
                po = pp2.tile([128, BLK_STEPS * WIN], f32, tag="p2")
                nc.tensor.matmul(
                    out=po[:, :colsi],
                    lhsT=w1_sb[:],
                    rhs=vt_sb[:, i0i * WIN : i0i * WIN + colsi],
                    start=True, stop=False,
                )
                nc.tensor.matmul(
                    out=po[:, :colsi], lhsT=w0_sb[:], rhs=x0i[:, :colsi],
                    start=False, stop=True,
                )
                if i in chunk_starts:
                    ot = op.tile(
                        [128, max(OUT_PLAN) * BLK_STEPS * WIN], bf16, tag="ot"
                    )
                    chunk_col0 = i0i * WIN
                    ot_cols = 0
                nc.scalar.activation(
                    out=ot[:, ot_cols : ot_cols + colsi],
                    in_=po[:, :colsi],
                    func=mybir.ActivationFunctionType.Identity,
                    bias=b_sb[:, 0:1],
                    scale=1.0,
                )
                ot_cols += colsi
                if i + 1 in chunk_starts or i == N_BLKS - 1:
                    nc.scalar.dma_start(
                        outT_d[:, chunk_col0 : chunk_col0 + ot_cols],
                        ot[:, :ot_cols],
                    )

            for j in range(N_BLKS):
                i0 = blk_i0[j]
                steps = BLOCK_PLAN[j]
                cols_blk = steps * WIN
                n_blk = blk_tot[j]
                na_b = blk_na[j]

                # one coalesced edge DMA per block (~0.5 MB)
                et = ep.tile([128, max_blk * 64], fp8, tag="et")
                nc.sync.dma_start(
                    et[:, : n_blk * 64], eh_d[:, off : off + n_blk, :]
                )
                if j == 1:
                    nc.scalar.dma_start(relr_sb[:], rel_d[:, rel_head:])
                # vT arrives in 5 chunks spread over the early blocks
                if j < 10 and j % 2 == 0:
                    k = j // 2
                    vc0 = k * (NPC // 5)
                    vc1 = NPC if k == 4 else (k + 1) * (NPC // 5)
                    nc.scalar.dma_start(vt_sb[:, vc0:vc1], vT_d[:, vc0:vc1])

                selb = sp.tile([128, max_blk * WIN], bf16, tag="selb")
                # rel2 dup-x2 keeps a dense 64-bit innermost run -> 2x uop
                if j < REL_HEAD_BLKS:
                    rel_sb, rb0 = relh_sb, 2 * off
                else:
                    rel_sb, rb0 = relr_sb, 2 * off - rel_head
                in0 = rel_sb[:, rb0 : rb0 + 2 * n_blk].rearrange(
                    "p (n one d) -> p n one d", one=1, d=2
                ).broadcast_to([128, n_blk, WIN // 2, 2])
                in1 = iota_sb[:].rearrange(
                    "p (w d) -> p w d", d=2
                ).rearrange(
                    "p (one w) d -> p one w d", one=1
                ).broadcast_to([128, n_blk, WIN // 2, 2])
                # (scalar_tensor_tensor would reach the DVE 4x_2p uop, but
                # walrus limits it to 3D APs and this pattern needs 4)
                nc.vector.tensor_tensor(
                    out=selb[:, : n_blk * WIN].rearrange(
                        "p (n w d) -> p n w d", w=WIN // 2, d=2
                    ),
                    in0=in0, in1=in1, op=mybir.AluOpType.is_equal,
                )

                x0 = x0p.tile([128, BLK_STEPS * WIN], bf16, tag="x0")
                ps = pp1.tile([128, BLK_STEPS * WIN], f32, tag="p1")
                t = 0
                for half, n_stp in ((0, na_step), (1, nb_step)):
                    r0 = half * 64
                    tt = 0
                    n_half = blk_na[j] if half == 0 else blk_nb[j]
                    for stp in range(steps):
                        for k in range(n_stp[i0 + stp]):
                            nc.tensor.matmul(
                                out=ps[
                                    r0 : r0 + 64,
                                    stp * WIN : (stp + 1) * WIN,
                                ],
                                lhsT=et[:, t * 64 : (t + 1) * 64],
                                rhs=selb[:, t * WIN : (t + 1) * WIN],
                                start=(tt == 0),
                                stop=(tt == n_half - 1),
                            )
                            t += 1
                            tt += 1
                nc.scalar.copy(x0[:, :cols_blk], ps[:, :cols_blk])
                off += n_blk

                if x0_prev is not None:
                    updater(j - 1, x0_prev, cols_prev, i0_prev)
                x0_prev, cols_prev, i0_prev = x0, cols_blk, i0
            updater(N_BLKS - 1, x0_prev, cols_prev, i0_prev)
    _split_multi_waits(nc)
    return nc


# ---------------------------------------------------------------------------
# public entry point
# ---------------------------------------------------------------------------


def kernel(vdata, edata_a, edata_b, conn_a, conn_b, W, b, _trace=False):
    in_maps, sched, node_lists = _preprocess(
        vdata, edata_a, edata_b, conn_a, conn_b, W, b
    )
    nc = _NC_CACHE.get(sched)
    if nc is None:
        nc = _build(sched)
        _NC_CACHE[sched] = nc
    kwargs = {}
    if _trace:
        kwargs = dict(trace=True, trace_cores=[0])
    res = run_bass_kernel_spmd(
        nc, in_maps, core_ids=list(range(N_CORES)), **kwargs
    )

    out_full = np.empty((NTOT, D_OUT), dtype=np.float32)
    for c in range(N_CORES):
        outT = res.results[c]["outT"].astype(np.float32)  # [128, NPC] bf16
        out_full[node_lists[c]] = outT.T
    out = out_full[:N_NODES]
    if _trace:
        return out, res
    return out
